# revision 1
# baseline (speedup 1.0000x reference)
"""Trainium2 Bass kernel for nn_CalculateHLayer (GNN message passing).

Computes, for adj [4096, 4096, 2] f32 and h [4096, 150] f32:
    A     = adj.sum(axis=2)          # [L, L]
    h_in  = A.T @ h                  # [L, D]
    h_out = A @ h                    # [L, D]
returning (h_in, h_out) as float32, matching the reference.

Distribution: adj is sharded row-wise (dim 0) across 8 NeuronCores, h is
replicated.  Each core computes its local rows of h_out directly and a
partial h_in (contraction over its local i rows); the 8 h_in partials are
summed on the host.

Per-core dataflow (Tile framework):
  - DMA adj row-stripe chunks [128 i, JC j, 2 e] f32 into SBUF.
  - DVE edge-sum adj[...,0] + adj[...,1] -> A tile bf16.
  - h_in:  matmul(psum, lhsT=A[i,j] tile, rhs=h_local[i,d]) accumulating
           over the 4 local i tiles, one PSUM bank per j tile, evacuated
           to DRAM each j tile.
  - h_out: PE-transpose each 128x128 A tile (identity matmul), then
           matmul(psum, lhsT=A_T[j,i], rhs=h[j,d]) accumulating over all
           32 j tiles in 4 persistent PSUM banks (one per local i tile).
Matmuls run in bf16 (inputs are O(1) magnitudes; PSUM accumulates fp32).
Output DMAs ride the scalar-engine HWDGE ring so they do not FIFO-block
the adjacency loads on the sync ring; h preloads ride gpsimd SWDGE.
"""

import sys

for _p in ("/opt/trn_rl_repo",):
    if _p not in sys.path:
        sys.path.append(_p)

from contextlib import ExitStack

import numpy as np

import concourse.bass as bass
import concourse.mybir as mybir
import concourse.tile as tile
from concourse import bacc
from concourse.bass_utils import run_bass_kernel_spmd
from concourse.masks import make_identity

L = 4096          # number of nodes
D = 150           # feature dim
NCORES = 8
R = L // NCORES   # rows of adj per core (512)
P = 128           # SBUF partitions
IT = R // P       # i tiles per core (4)
JT = L // P       # j tiles (32)

F32 = mybir.dt.float32
BF16 = mybir.dt.bfloat16

# Best-known config (used by kernel()).
DEFAULT_CFG = dict(
    jc=512,            # j-chunk width per adj DMA
    out_ring="scalar",  # engine for output DMAs
    pre_ring="gpsimd",  # engine for h preload DMAs
    hin_bf16=False,     # write h_in partials as bf16
    adj_bufs=6,
    # stage toggles (bench attribution; all True for the real kernel)
    esum=True,
    hin=True,
    trans=True,
    hout=True,
    do_out=True,
    fake_notrans=False,  # hout matmuls use untransposed A (perf-only, WRONG result)
    trans_via="pe",      # "pe" | "dma"
    loadonly=False,      # adj DMA only (bench)
    # pipeline tuning
    pack_phout=True,     # pack the 4 h_out accumulators into 2 PSUM banks
    psum_hin_bufs=4,     # PSUM double/quad buffering for the h_in groups
    psum_tr_bufs=2,
    evac_engine="any",   # engine for PSUM->SBUF evac copies
    hin_pack=2,          # j-tiles packed per h_in PSUM bank (1..3)
    tr_pack=True,        # pack the 4 transposes of a j-tile into one PSUM bank
    do_dma=True,         # emit output DMAs (False isolates evac copies in bench)
    sbuf_accum=True,     # stage outputs in SBUF; few big output DMAs
    out_groups=4,        # output DMA granularity (j-tile groups) when sbuf_accum
    out_dbuf=True,       # double-buffer the SBUF output staging across loop iters
    load_split=False,    # alternate adj loads between sync and scalar HWDGE rings
    writeonly=False,     # bench: only the output DMAs, no loads/compute
)

_NC_CACHE = {}


def _build(loop_k=None, **overrides):
    """Build the per-core Bass program.

    loop_k: if set, wrap the compute body in a hardware For loop repeating it
    loop_k times (device-time microbenchmarking; the body is idempotent).
    """
    cfg = dict(DEFAULT_CFG)
    cfg.update(overrides)
    key = (loop_k, tuple(sorted(cfg.items())))
    if key in _NC_CACHE:
        return _NC_CACHE[key]

    JC = cfg["jc"]
    NJC = L // JC
    JPC = JC // P
    hin_dt = BF16 if cfg["hin_bf16"] else F32

    nc = bacc.Bacc()
    adj = nc.declare_dram_parameter("adj", [R, L, 2], F32, isOutput=False)
    h = nc.declare_dram_parameter("h", [L, D], F32, isOutput=False)
    hloc = nc.declare_dram_parameter("hloc", [R, D], F32, isOutput=False)
    hin = nc.declare_dram_parameter("hin", [L, D], hin_dt, isOutput=True)
    hout = nc.declare_dram_parameter("hout", [R, D], F32, isOutput=True)

    out_eng = getattr(nc, cfg["out_ring"])
    pre_eng = getattr(nc, cfg["pre_ring"])

    with ExitStack() as ctx:
        tc = ctx.enter_context(tile.TileContext(nc))
        const = ctx.enter_context(tc.tile_pool(name="const", bufs=1))
        stage = ctx.enter_context(tc.tile_pool(name="stage", bufs=1))
        adjp = ctx.enter_context(tc.tile_pool(name="adjp", bufs=cfg["adj_bufs"]))
        abfp = ctx.enter_context(tc.tile_pool(name="abfp", bufs=2))
        atp = ctx.enter_context(tc.tile_pool(name="atp", bufs=4))
        evp = ctx.enter_context(tc.tile_pool(name="evp", bufs=4))
        ps_hin = ctx.enter_context(
            tc.tile_pool(name="ps_hin", bufs=cfg["psum_hin_bufs"], space="PSUM")
        )
        ps_tr = ctx.enter_context(
            tc.tile_pool(name="ps_tr", bufs=cfg["psum_tr_bufs"], space="PSUM")
        )
        ps_hout = ctx.enter_context(tc.tile_pool(name="ps_hout", bufs=1, space="PSUM"))

        ident = const.tile([P, P], BF16)
        make_identity(nc, ident)

        # DRAM views tiled to 128 partitions (row = o*128 + p)
        h_t = h.rearrange("(o p) d -> p o d", p=P)          # [128, 32, 150]
        hloc_t = hloc.rearrange("(o p) d -> p o d", p=P)    # [128, 4, 150]
        hin_t = hin.rearrange("(o p) d -> p o d", p=P)
        hout_t = hout.rearrange("(o p) d -> p o d", p=P)
        adj_t = adj.rearrange("(io p) l e -> io p (l e)", p=P)  # [4, 128, 8192]

        # Preload h (replicated) and the core's local h rows; cast to bf16.
        hf = stage.tile([P, JT, D], F32)
        pre_eng.dma_start(hf, h_t)
        hbf = const.tile([P, JT, D], BF16)
        nc.any.tensor_copy(hbf, hf)

        hlf = stage.tile([P, IT, D], F32)
        pre_eng.dma_start(hlf, hloc_t)
        hlbf = const.tile([P, IT, D], BF16)
        nc.any.tensor_copy(hlbf, hlf)

        # Persistent PSUM accumulators for the core's 4 h_out row tiles.
        if cfg["pack_phout"]:
            # Two [P, 2D] banks, each holding two j-accumulators side by side
            # ([P, 300] f32 = 1200 B/partition fits one 2 KB PSUM bank).
            pairs = [ps_hout.tile([P, 2 * D], F32, name=f"phoutp{p}") for p in range(2)]
            phout = [pairs[it // 2][:, (it % 2) * D : (it % 2 + 1) * D] for it in range(IT)]
        else:
            phout = [ps_hout.tile([P, D], F32, name=f"phout{it}") for it in range(IT)]

        def evac_copy(dst, src):
            eng = cfg["evac_engine"]
            if eng == "any":
                nc.any.tensor_copy(dst, src)
            elif eng == "scalar":
                nc.scalar.copy(dst, src)
            else:
                getattr(nc, eng).tensor_copy(dst, src)

        # SBUF staging for outputs: PSUM->SBUF copies are cheap, but each
        # output DMA has ~1us of serialized fixed cost — so stage everything
        # in SBUF and emit only a handful of large output DMAs.
        outsb = ctx.enter_context(
            tc.tile_pool(name="outsb", bufs=2 if cfg["out_dbuf"] else 1)
        )

        def body():
            if cfg["sbuf_accum"]:
                hin_sb = outsb.tile([P, JT, D], hin_dt, tag="hin_sb")
                hout_sb = outsb.tile([P, IT, D], F32, tag="hout_sb")
            else:
                hin_sb = hout_sb = None
            if cfg["writeonly"]:
                nc.gpsimd.memset(hin_sb, 0.0)
                nc.gpsimd.memset(hout_sb, 0.0)
                gsz = JT // cfg["out_groups"]
                for g in range(cfg["out_groups"]):
                    out_eng.dma_start(
                        hin_t[:, g * gsz : (g + 1) * gsz, :],
                        hin_sb[:, g * gsz : (g + 1) * gsz, :],
                    )
                out_eng.dma_start(hout_t, hout_sb)
                if not cfg["loadonly"]:
                    return
                # else fall through to the load loop (load+write bench)
            for jc_i in range(NJC):
                a_bf = []
                for it in range(IT):
                    adj_sb = adjp.tile([P, JC * 2], F32, tag="adj")
                    load_eng = (
                        (nc.sync if it % 2 == 0 else nc.scalar)
                        if cfg["load_split"]
                        else nc.sync
                    )
                    load_eng.dma_start(
                        adj_sb, adj_t[it, :, jc_i * JC * 2 : (jc_i + 1) * JC * 2]
                    )
                    if cfg["loadonly"] or not cfg["esum"]:
                        continue
                    ab = abfp.tile([P, JC], BF16, tag=f"abf{it}")
                    av = adj_sb.rearrange("p (j e) -> p j e", e=2)
                    nc.vector.tensor_add(ab, av[:, :, 0], av[:, :, 1])
                    a_bf.append(ab)

                if cfg["loadonly"] or not cfg["esum"]:
                    continue

                hp = cfg["hin_pack"]
                for j8 in range(JPC):
                    jt = jc_i * JPC + j8
                    jsl = bass.ts(j8, P)

                    if cfg["hin"]:
                        # h_in[j-tile] = sum_it A[it, j-tile].T @ h_local[it]
                        # hp j-tiles share one PSUM bank; evac once per bank.
                        sub = jt % hp
                        if sub == 0:
                            pin_bank = ps_hin.tile([P, hp * D], F32, tag="phin")
                            body.pin_bank = pin_bank
                        pin = body.pin_bank[:, sub * D : (sub + 1) * D]
                        last_in_bank = sub == hp - 1 or jt == JT - 1
                        for it in range(IT):
                            # start=True clears the whole PSUM zero-region, so
                            # only the bank's first matmul may set it; co-packed
                            # slices overwrite via per-element has_written bits.
                            nc.tensor.matmul(
                                pin,
                                lhsT=a_bf[it][:, jsl],
                                rhs=hlbf[:, it, :],
                                start=(sub == 0 and it == 0),
                                stop=(last_in_bank and it == IT - 1),
                            )
                        if cfg["do_out"] and last_in_bank:
                            w = sub + 1
                            src = body.pin_bank.rearrange("p (s d) -> p s d", s=hp)
                            if cfg["sbuf_accum"]:
                                evac_copy(
                                    hin_sb[:, jt - w + 1 : jt + 1, :], src[:, :w, :]
                                )
                                gsz = JT // cfg["out_groups"]
                                if cfg["do_dma"] and (jt + 1) % gsz == 0:
                                    g0 = jt + 1 - gsz
                                    out_eng.dma_start(
                                        hin_t[:, g0 : jt + 1, :],
                                        hin_sb[:, g0 : jt + 1, :],
                                    )
                            else:
                                ev = evp.tile([P, hp, D], hin_dt, tag="ev_hin")
                                evac_copy(ev[:, :w, :], src[:, :w, :])
                                if cfg["do_dma"]:
                                    out_eng.dma_start(
                                        hin_t[:, jt - w + 1 : jt + 1, :], ev[:, :w, :]
                                    )

                    # h_out[it] += A[it, j-tile] @ h[j-tile]
                    if cfg["trans"] and not cfg["fake_notrans"] and cfg["tr_pack"]:
                        ptr4 = ps_tr.tile([P, IT * P], BF16, tag="ptr")
                        for it in range(IT):
                            nc.tensor.matmul(
                                ptr4[:, bass.ts(it, P)],
                                a_bf[it][:, jsl],
                                ident,
                                is_transpose=True,
                                start=(it == 0),
                                stop=(it == IT - 1),
                            )
                        at4 = atp.tile([P, IT * P], BF16, tag="at")
                        nc.any.tensor_copy(at4, ptr4)
                        ats = [at4[:, bass.ts(it, P)] for it in range(IT)]
                    elif cfg["trans"] and not cfg["fake_notrans"]:
                        ats = []
                        for it in range(IT):
                            if cfg["trans_via"] == "dma":
                                at2 = atp.tile([P, P], BF16, tag="at")
                                nc.scalar.dma_start(
                                    at2, a_bf[it][:, jsl], transpose=True
                                )
                            else:
                                ptr = ps_tr.tile([P, P], BF16, tag="ptr")
                                nc.tensor.transpose(ptr, a_bf[it][:, jsl], ident)
                                at2 = atp.tile([P, P], BF16, tag="at")
                                nc.any.tensor_copy(at2, ptr)
                            ats.append(at2)
                    elif cfg["fake_notrans"]:
                        ats = [a_bf[it][:, jsl] for it in range(IT)]
                    else:
                        ats = None
                    if cfg["hout"] and ats is not None:
                        for it in range(IT):
                            if cfg["pack_phout"]:
                                # paired accumulators share a bank: only the
                                # bank's first write may set start (zero-region
                                # clear); its last write sets stop.
                                mm_start = jt == 0 and it % 2 == 0
                                mm_stop = jt == JT - 1 and it % 2 == 1
                            else:
                                mm_start = jt == 0
                                mm_stop = jt == JT - 1
                            nc.tensor.matmul(
                                phout[it],
                                lhsT=ats[it],
                                rhs=hbf[:, jt, :],
                                start=mm_start,
                                stop=mm_stop,
                            )

            if not cfg["loadonly"] and cfg["esum"] and cfg["hout"] and cfg["do_out"] and (
                cfg["trans"] or cfg["fake_notrans"]
            ):
                if cfg["sbuf_accum"]:
                    for it in range(IT):
                        evac_copy(hout_sb[:, it, :], phout[it])
                    if cfg["do_dma"]:
                        out_eng.dma_start(hout_t, hout_sb)
                else:
                    for it in range(IT):
                        ev = evp.tile([P, D], F32, tag="ev_hout")
                        evac_copy(ev, phout[it])
                        out_eng.dma_start(hout_t[:, it, :], ev)

        if loop_k is None:
            body()
        else:
            with tc.For_i(0, loop_k, 1):
                body()

    nc.compile()
    _NC_CACHE[key] = nc
    return nc


def _run(adj, h, trace=False, loop_k=None, **overrides):
    nc = _build(loop_k=loop_k, **overrides)
    in_maps = []
    for c in range(NCORES):
        sl = slice(c * R, (c + 1) * R)
        in_maps.append(
            {
                "adj": np.ascontiguousarray(adj[sl]),
                "h": h,
                "hloc": np.ascontiguousarray(h[sl]),
            }
        )
    return run_bass_kernel_spmd(nc, in_maps, list(range(NCORES)), trace=trace)


def kernel(**inputs):
    adj = np.ascontiguousarray(
        np.asarray(inputs["unpreprocessed_unweight_adj_matrix"], dtype=np.float32)
    )
    h = np.ascontiguousarray(np.asarray(inputs["h"], dtype=np.float32))

    res = _run(adj, h)
    outs = res.results
    h_in = np.zeros((L, D), np.float32)
    for c in range(NCORES):
        h_in += np.asarray(outs[c]["hin"], dtype=np.float32)
    h_out = np.concatenate([outs[c]["hout"] for c in range(NCORES)], axis=0)
    return (h_in, h_out)

